# revision 7
# baseline (speedup 1.0000x reference)
"""GAT (2-layer) Bass kernel for 8 TRN2 cores — v2.

Design (v2, derived from trace analysis of v1):
- Bottleneck was SWDGE descriptor generation (GpSimd) — ~8ns/descriptor,
  ~500k descriptors (hg + ad gathers x 2 layers).
- v2 assigns each destination node to a fixed SBUF partition lane
  (lane = dst) with nodes degree-sorted into blocks so every block's
  slab count ~= its max in-degree. Per-edge alpha_dst becomes an affine
  slice of the block's self-loop rows; the ad/ad2 gathers, the one-hot
  masks and the PE aggregation matmuls all disappear. Aggregation is a
  DVE free-dim reduce over slabs.
- Single h-table (no A/B split): gathers fetch PAIRS of rows
  (elem = 2 rows, idx = row//2 fits int16) and a host-provided parity
  mask folds the half-select into the ee multiply.
- Gathers batched into CS-slab chunks and spread over parallel SWDGE
  queues (num_swdge_queues); measured 1.7x from 2 queues alone.
"""
import os
import sys
import math

import numpy as np
import ml_dtypes


def _setup_paths():
    for p in ("/opt/trn_rl_repo", "/root/.axon_site/_ro/trn_rl_repo"):
        if os.path.isdir(p) and p not in sys.path:
            sys.path.insert(0, p)


_setup_paths()

import concourse.bass as bass  # noqa: E402
import concourse.mybir as mybir  # noqa: E402
import concourse.tile as tile  # noqa: E402
from concourse import bacc, bass_utils  # noqa: E402

bf16 = ml_dtypes.bfloat16
BF = mybir.dt.bfloat16
F32 = mybir.dt.float32
I16 = mybir.dt.int16
AL = mybir.AluOpType
AF = mybir.ActivationFunctionType
AX = mybir.AxisListType

NQ = 4          # SWDGE queues
CS = 16         # slabs per gather chunk


class Cfg:
    def __init__(self):
        self.N, self.E = 50000, 800000
        self.IN_C, self.HID, self.OUT_C, self.HEADS = 128, 64, 64, 4
        self.NCLS, self.NEG, self.NCORES = 40, 0.2, 8
        self.NB = self.N // self.NCORES            # 6250
        self.NBLK = 49
        self.NDP = self.NBLK * 128                 # 6272
        self.RTOT = self.NDP * self.NCORES         # 50176
        self.NPAIR = self.RTOT // 2                # 25088
        self.C1 = self.HEADS * self.HID            # 256
        self.T1W = 384                             # t1 row cols (264 used)
        self.T2W = 128                             # cc3 row cols (66 used)


def _pack_idx(flat):
    """flat [n] -> dma_gather index layout [128, n//16] int16."""
    n = flat.shape[0]
    assert n % 16 == 0
    a = flat.reshape(n // 16, 16).T                # [16, n/16]
    return np.tile(a, (8, 1)).astype(np.int16)     # [128, n/16]


def host_prep(cfg, x, edge_index, W1, att_src1, att_dst1, b1, W2, att_src2,
              att_dst2, b2):
    c = cfg
    src = np.asarray(edge_index[0], dtype=np.int64)
    dst = np.asarray(edge_index[1], dtype=np.int64)

    # ---- degree-sorted node assignment ----
    deg = np.zeros(c.RTOT, np.int64)
    np.add.at(deg, dst, 1)
    deg[:c.N] += 1                                  # self-loop
    order = np.argsort(-deg, kind="stable")         # rank -> node
    rank = np.empty(c.RTOT, np.int64)
    rank[order] = np.arange(c.RTOT)
    stratum = rank // 1024                          # = block, 49 strata
    core = rank % 8
    lane = (rank // 8) % 128
    row_of = core * c.NDP + stratum * 128 + lane    # node -> padded row

    deg_sorted = deg[order]
    Kb = np.maximum(1, deg_sorted[::1024][:c.NBLK].copy())  # max deg / stratum
    slab_base = np.zeros(c.NBLK + 1, np.int64)
    slab_base[1:] = np.cumsum(Kb - 1)
    SLB = int(slab_base[-1])                        # gathered slabs per core

    # ---- per-edge slotting (lane = dst lane, slab j>=1 in arrival order) ---
    dcore = core[dst]
    dblk = stratum[dst]
    dlane = lane[dst]
    gkey = row_of[dst]                              # unique per (core,blk,lane)
    order2 = np.argsort(gkey, kind="stable")
    gk = gkey[order2]
    starts = np.searchsorted(gk, np.arange(c.RTOT))
    j = np.empty(c.E, np.int64)
    j[order2] = np.arange(c.E) - starts[gk] + 1     # 1..deg-1
    assert (j < deg[dst]).all() and (j >= 1).all()

    srow = row_of[src]
    pairv = srow // 2
    par = srow % 2

    slab = slab_base[dblk] + (j - 1)
    pairidx = np.zeros((c.NCORES, SLB, 128), np.int64)
    par0 = np.zeros((c.NCORES, SLB, 128), np.float32)
    par1 = np.zeros((c.NCORES, SLB, 128), np.float32)
    pairidx[dcore, slab, dlane] = pairv
    par0[dcore, slab, dlane] = (par == 0)
    par1[dcore, slab, dlane] = (par == 1)

    # ---- augmented weights (h | a_src | a_dst) ----
    W1 = np.asarray(W1, np.float32)
    a_s1 = np.asarray(att_src1, np.float32).reshape(c.HEADS, c.HID)
    a_d1 = np.asarray(att_dst1, np.float32).reshape(c.HEADS, c.HID)
    W1r = W1.reshape(c.IN_C, c.HEADS, c.HID)
    w1aug = np.zeros((c.IN_C, c.C1 + 8), np.float32)
    w1aug[:, :c.C1] = W1
    w1aug[:, c.C1:c.C1 + 4] = np.einsum("khc,hc->kh", W1r, a_s1)
    w1aug[:, c.C1 + 4:c.C1 + 8] = np.einsum("khc,hc->kh", W1r, a_d1)

    W2 = np.asarray(W2, np.float32)
    a_s2 = np.asarray(att_src2, np.float32).reshape(c.OUT_C)
    a_d2 = np.asarray(att_dst2, np.float32).reshape(c.OUT_C)
    w2aug = np.zeros((c.C1, 66), np.float32)
    w2aug[:, :c.OUT_C] = W2
    w2aug[:, c.OUT_C] = W2 @ a_s2
    w2aug[:, c.OUT_C + 1] = W2 @ a_d2

    assert np.allclose(np.asarray(b1), 0) and np.allclose(np.asarray(b2), 0)

    # ---- x tiles (row-mapped, transposed for PE) ----
    x = np.asarray(x, np.float32)
    x_pad = np.zeros((c.RTOT, c.IN_C), np.float32)
    x_pad[row_of[:c.N]] = x
    NT1 = c.RTOT // 128
    xT = x_pad.reshape(NT1, 128, c.IN_C).transpose(0, 2, 1)   # [t, k, n]
    xT8 = np.ascontiguousarray(
        xT.reshape(49, 8, c.IN_C, 128).transpose(0, 2, 1, 3)).astype(bf16)

    in_maps = []
    for ci in range(c.NCORES):
        xo = x_pad[ci * c.NDP:(ci + 1) * c.NDP]
        xoT = xo.reshape(c.NBLK, 128, c.IN_C).transpose(0, 2, 1)
        xoT7 = np.ascontiguousarray(
            xoT.reshape(7, 7, c.IN_C, 128).transpose(0, 2, 1, 3)).astype(bf16)
        in_maps.append({
            "x_t3": xT8,
            "x_own": xoT7,
            "w1aug": w1aug.astype(bf16),
            "w2aug": np.ascontiguousarray(
                w2aug.astype(bf16).reshape(2, 128, 66).transpose(1, 0, 2)),
            "idx": _pack_idx(pairidx[ci].reshape(-1)),       # [128, SLB*8]
            "par01": np.ascontiguousarray(np.stack(
                [par0[ci].T, par1[ci].T], axis=1).astype(bf16)),  # [128,2,SLB]
            "ones": np.ones((128, 1), np.float32),
        })
    meta = dict(SLB=SLB, Kb=Kb, slab_base=slab_base, row_of=row_of)
    return in_maps, meta


def emulate(cfg, in_maps, meta, fc_w, fc_b):
    """Numpy emulation of the device program (for correctness checks)."""
    c = cfg
    SLB = meta["SLB"]
    Kb = meta["Kb"]
    slab_base = meta["slab_base"]
    pools = []
    for ci in range(c.NCORES):
        m = in_maps[ci]
        # phase A: full table
        xT = m["x_t3"].astype(np.float32).transpose(0, 2, 1, 3).reshape(
            392, c.IN_C, 128)
        w1 = m["w1aug"].astype(np.float32)
        t1 = np.zeros((c.RTOT, c.T1W), np.float32)
        for t in range(392):
            h = (xT[t].T @ w1).astype(bf16).astype(np.float32)
            t1[t * 128:(t + 1) * 128, :c.C1 + 8] = h
        xoT = m["x_own"].astype(np.float32).transpose(0, 2, 1, 3).reshape(
            c.NBLK, c.IN_C, 128)
        hself = np.zeros((c.NBLK, 128, c.C1 + 8), np.float32)
        for b in range(c.NBLK):
            hself[b] = (xoT[b].T @ w1).astype(bf16).astype(np.float32)
        t1p = t1.reshape(c.NPAIR, 2 * c.T1W)
        # unpack idx
        idx = m["idx"][:16]                             # [16, SLB*8]
        n = SLB * 128
        flat2 = np.empty(n, np.int64)
        flat2[:] = idx[np.arange(n) % 16, np.arange(n) // 16]
        pairs = flat2.reshape(SLB, 128)
        par0 = m["par01"][:, 0].astype(np.float32).T     # [SLB, 128]
        par1 = m["par01"][:, 1].astype(np.float32).T
        # phase B
        h1d = np.zeros((c.NDP, c.C1), np.float32)
        for b in range(c.NBLK):
            s0, s1 = slab_base[b], slab_base[b + 1]
            hs = hself[b]                                # [128, 264]
            asf = hs[:, c.C1:c.C1 + 4]
            adf = hs[:, c.C1 + 4:c.C1 + 8]
            z = asf + adf
            ee = np.exp(np.maximum(z, c.NEG * z))
            num = ee[:, :, None] * hs[:, None, :c.C1].reshape(128, c.HEADS, c.HID)
            num = num.reshape(128, c.C1)
            den = ee.copy()
            for s in range(s0, s1):
                rows = t1p[pairs[s]]                     # [128, 768]
                zA = rows[:, c.C1:c.C1 + 4] + adf
                zB = rows[:, c.T1W + c.C1:c.T1W + c.C1 + 4] + adf
                eeA = np.exp(np.maximum(zA, c.NEG * zA)).astype(bf16).astype(np.float32) * par0[s][:, None]
                eeB = np.exp(np.maximum(zB, c.NEG * zB)).astype(bf16).astype(np.float32) * par1[s][:, None]
                vA = (eeA[:, :, None] * rows[:, :c.C1].reshape(128, 4, 64)).reshape(128, c.C1)
                vB = (eeB[:, :, None] * rows[:, c.T1W:c.T1W + c.C1].reshape(128, 4, 64)).reshape(128, c.C1)
                num += (vA.astype(bf16).astype(np.float32)
                        + vB.astype(bf16).astype(np.float32))
                den += eeA + eeB
            h1 = num.reshape(128, 4, 64) / (den[:, :, None] + 1e-16)
            h1d[b * 128:(b + 1) * 128] = np.maximum(
                h1.reshape(128, c.C1), 0).astype(bf16).astype(np.float32)
        # phase C
        w2 = m["w2aug"].astype(np.float32).transpose(1, 0, 2).reshape(c.C1, 66)
        h2pre = (h1d @ w2).astype(bf16).astype(np.float32)  # [NDP, 66]
        pools.append(h2pre)
    # phase D: allgather
    cc3 = np.concatenate(pools, axis=0)                  # [RTOT, 66]
    cc3_128 = np.zeros((c.RTOT, c.T2W), np.float32)
    cc3_128[:, :66] = cc3
    cc3p = cc3_128.reshape(c.NPAIR, 2 * c.T2W)
    # phase E
    tot = np.zeros(c.OUT_C, np.float64)
    for ci in range(c.NCORES):
        m = in_maps[ci]
        idx = m["idx"][:16]
        n = SLB * 128
        flat2 = np.empty(n, np.int64)
        flat2[:] = idx[np.arange(n) % 16, np.arange(n) // 16]
        pairs = flat2.reshape(SLB, 128)
        par0 = m["par01"][:, 0].astype(np.float32).T
        par1 = m["par01"][:, 1].astype(np.float32).T
        h2self = cc3_128[ci * c.NDP:(ci + 1) * c.NDP]
        for b in range(c.NBLK):
            s0, s1 = slab_base[b], slab_base[b + 1]
            hs = h2self[b * 128:(b + 1) * 128]
            z = hs[:, 64] + hs[:, 65]
            ee = np.exp(np.maximum(z, c.NEG * z))
            num = ee[:, None] * hs[:, :64]
            den = ee.copy()
            for s in range(s0, s1):
                rows = cc3p[pairs[s]]                    # [128, 256]
                zA = rows[:, 64] + hs[:, 65]
                zB = rows[:, c.T2W + 64] + hs[:, 65]
                eeA = np.exp(np.maximum(zA, c.NEG * zA)).astype(bf16).astype(np.float32) * par0[s]
                eeB = np.exp(np.maximum(zB, c.NEG * zB)).astype(bf16).astype(np.float32) * par1[s]
                num += (eeA[:, None] * rows[:, :64]
                        + eeB[:, None] * rows[:, c.T2W:c.T2W + 64])
                den += eeA + eeB
            o2 = np.maximum(num / (den[:, None] + 1e-16), 0)
            tot += o2.sum(axis=0)
    pooled = (tot / c.N).astype(np.float32)
    logits = pooled @ np.asarray(fc_w, np.float32) + np.asarray(fc_b, np.float32)
    mx = logits.max()
    return (logits - (mx + np.log(np.exp(logits - mx).sum()))).reshape(1, -1)


def build(cfg, SLB, slab_base, debug=False):
    c = cfg
    nc = bacc.Bacc("TRN2", target_bir_lowering=False, debug=False,
                   num_devices=c.NCORES, num_swdge_queues=NQ)

    # ---- IO ----
    x_t3_d = nc.dram_tensor("x_t3", [49, c.IN_C, 8, 128], BF, kind="ExternalInput").ap()
    x_own_d = nc.dram_tensor("x_own", [7, c.IN_C, 7, 128], BF, kind="ExternalInput").ap()
    w1_d = nc.dram_tensor("w1aug", [c.IN_C, c.C1 + 8], BF, kind="ExternalInput").ap()
    w2_d = nc.dram_tensor("w2aug", [128, 2, 66], BF, kind="ExternalInput").ap()
    idx_d = nc.dram_tensor("idx", [128, SLB * 8], I16, kind="ExternalInput").ap()
    par01_d = nc.dram_tensor("par01", [128, 2, SLB], BF, kind="ExternalInput").ap()
    ones_d = nc.dram_tensor("ones", [128, 1], F32, kind="ExternalInput").ap()
    pool_d = nc.dram_tensor("pool64", [c.OUT_C, 1], F32, kind="ExternalOutput").ap()
    if debug:
        h1dbg_d = nc.dram_tensor("h1dbg", [c.NDP, c.C1], F32, kind="ExternalOutput").ap()

    # ---- internal DRAM ----
    t1 = nc.dram_tensor("t1", [c.NPAIR, 2, c.T1W], BF, kind="Internal").ap()
    h1d = nc.dram_tensor("h1d", [c.NDP, c.C1], BF, kind="Internal").ap()
    cc3in = nc.dram_tensor("cc3in", [c.NDP, c.T2W], BF, kind="Internal").ap()
    cc3 = nc.dram_tensor("cc3", [c.NPAIR, 2, c.T2W], BF, kind="Internal",
                         addr_space="Shared").ap()

    t1_rows = t1.rearrange("p a b -> (p a) b")          # [RTOT, 384]
    t1_pairs = t1.rearrange("p a b -> p (a b)")         # [NPAIR, 768]
    cc3_rows = cc3.rearrange("p a b -> (p a) b")
    cc3_pairs = cc3.rearrange("p a b -> p (a b)")

    # chunk schedule
    chunks = []
    s = 0
    while s < SLB:
        chunks.append((s, min(s + CS, SLB)))
        s = min(s + CS, SLB)
    blk_ranges = [[] for _ in range(c.NBLK)]
    for cid, (c0, c1) in enumerate(chunks):
        for b in range(c.NBLK):
            lo = max(int(slab_base[b]), c0)
            hi = min(int(slab_base[b + 1]), c1)
            if lo < hi:
                blk_ranges[b].append((cid, lo - c0, hi - c0))

    with tile.TileContext(nc) as tc:
        with tc.tile_pool(name="const", bufs=1) as cpool, \
             tc.tile_pool(name="pa", bufs=2) as pa, \
             tc.tile_pool(name="pp", bufs=2, space="PSUM") as pp, \
             tc.tile_pool(name="pg", bufs=2) as pg, \
             tc.tile_pool(name="pge", bufs=4) as pge, \
             tc.tile_pool(name="pv", bufs=1) as pv, \
             tc.tile_pool(name="acc", bufs=2) as accp, \
             tc.tile_pool(name="sm", bufs=4) as sm:

            w1s = cpool.tile_from(w1_d)                  # [128, 264]
            w2s = cpool.tile_from(w2_d)                  # [128, 2, 66]
            idx_s = cpool.tile_from(idx_d)               # [128, SLB*8]
            par_s = cpool.tile_from(par01_d)             # [128, 2, SLB]
            ones_s = cpool.tile_from(ones_d)
            hselfA = cpool.tile([128, c.NBLK, c.C1 + 8], BF)
            h2self = cpool.tile([128, c.NBLK, c.T2W], BF)

            # ============ phase A: replicated h table + own rows ============
            for g in range(49):
                xg = pa.tile([128, 8, 128], BF, tag="xg")
                nc.sync.dma_start(out=xg[:], in_=x_t3_d[g, :, :, :])
                hb8 = pa.tile([128, 8, c.C1 + 8], BF, tag="hb8")
                for i in range(8):
                    ps = pp.tile([128, c.C1 + 8], F32, tag="A")
                    nc.tensor.matmul(out=ps[:], lhsT=xg[:, i, :], rhs=w1s[:],
                                     start=True, stop=True)
                    nc.scalar.activation(out=hb8[:, i, :], in_=ps[:], func=AF.Copy)
                nc.sync.dma_start(
                    out=t1_rows[g * 1024:(g + 1) * 1024, 0:c.C1 + 8].rearrange(
                        "(a p) n -> p a n", p=128),
                    in_=hb8[:])
            for g in range(7):
                xo = pa.tile([128, 7, 128], BF, tag="xo")
                nc.sync.dma_start(out=xo[:], in_=x_own_d[g, :, :, :])
                for i in range(7):
                    pso = pp.tile([128, c.C1 + 8], F32, tag="A")
                    nc.tensor.matmul(out=pso[:], lhsT=xo[:, i, :], rhs=w1s[:],
                                     start=True, stop=True)
                    nc.scalar.activation(out=hselfA[:, g * 7 + i, :], in_=pso[:],
                                         func=AF.Copy)

            # ============ phase B: layer-1 aggregation ============
            hgt = {}
            for b in range(c.NBLK):
                zs = sm.tile([128, 4], F32, tag="zs")
                nc.vector.tensor_tensor(out=zs[:], in0=hselfA[:, b, c.C1:c.C1 + 4],
                                        in1=hselfA[:, b, c.C1 + 4:c.C1 + 8],
                                        op=AL.add)
                lrs = sm.tile([128, 4], F32, tag="lrs")
                nc.vector.scalar_tensor_tensor(
                    out=lrs[:], in0=zs[:], scalar=c.NEG, in1=zs[:],
                    op0=AL.mult, op1=AL.max)
                ees = sm.tile([128, 4], BF, tag="ees")
                nc.scalar.activation(out=ees[:], in_=lrs[:], func=AF.Exp)
                num = accp.tile([128, c.C1], F32, tag="num")
                den = accp.tile([128, 4], F32, tag="den")
                nc.scalar.activation(out=den[:], in_=ees[:], func=AF.Copy)
                nc.vector.tensor_tensor(
                    out=num[:].rearrange("p (h q) -> p h q", h=4),
                    in0=hselfA[:, b, 0:c.C1].rearrange("p (h q) -> p h q", h=4),
                    in1=ees[:, :, None].to_broadcast([128, 4, c.HID]),
                    op=AL.mult)

                for (cid, j0, j1) in blk_ranges[b]:
                    ns = j1 - j0
                    c0, c1_ = chunks[cid]
                    ncs = c1_ - c0
                    for g_ in (cid, cid + 1):
                        if g_ < len(chunks) and g_ not in hgt:
                            g0, g1 = chunks[g_]
                            gn = g1 - g0
                            hgx = pg.tile([128, CS, 2 * c.T1W], BF, tag="hg")
                            nc.gpsimd.dma_gather(
                                out_ap=hgx[:, 0:gn, :], in_ap=t1_pairs[:, :],
                                idxs_ap=idx_s[:, g0 * 8:g1 * 8],
                                num_idxs=gn * 128, num_idxs_reg=gn * 128,
                                elem_size=2 * c.T1W, single_packet=False,
                                queue_num=g_ % NQ)
                            hgt[g_] = hgx
                    hg = hgt[cid]
                    hgr = hg[:, j0:j1, :].rearrange("p j (a w) -> p a j w", a=2)
                    # z for both halves in one op: [128, 2, ns, 4]
                    zz = sm.tile([128, 2, CS, 4], F32, tag="zz")
                    nc.vector.tensor_tensor(
                        out=zz[:, :, 0:ns, :], in0=hgr[:, :, :, c.C1:c.C1 + 4],
                        in1=hselfA[:, b, None, None, c.C1 + 4:c.C1 + 8]
                        .to_broadcast([128, 2, ns, 4]), op=AL.add)
                    nc.vector.scalar_tensor_tensor(
                        out=zz[:, :, 0:ns, :], in0=zz[:, :, 0:ns, :], scalar=c.NEG,
                        in1=zz[:, :, 0:ns, :], op0=AL.mult, op1=AL.max)
                    ee = sm.tile([128, 2, CS, 4], BF, tag="ee")
                    nc.scalar.activation(out=ee[:, :, 0:ns, :],
                                         in_=zz[:, :, 0:ns, :], func=AF.Exp)
                    nc.vector.tensor_tensor(
                        out=ee[:, :, 0:ns, :], in0=ee[:, :, 0:ns, :],
                        in1=par_s[:, :, c0 + j0:c0 + j1, None].to_broadcast(
                            [128, 2, ns, 4]), op=AL.mult)
                    v = pv.tile([128, 2, CS, c.C1], BF, tag="v")
                    for a in range(2):
                        nc.vector.tensor_tensor(
                            out=v[:, a, 0:ns, :].rearrange(
                                "p j (h q) -> p j h q", h=4),
                            in0=hg[:, j0:j1, a * c.T1W:a * c.T1W + c.C1]
                            .rearrange("p j (h q) -> p j h q", h=4),
                            in1=ee[:, a, 0:ns, :, None].to_broadcast(
                                [128, ns, 4, c.HID]), op=AL.mult)
                    vs = pv.tile([128, CS, c.C1], BF, tag="vs")
                    nc.vector.tensor_tensor(
                        out=vs[:, 0:ns, :], in0=v[:, 0, 0:ns, :],
                        in1=v[:, 1, 0:ns, :], op=AL.add)
                    nr = ns
                    if ns % 2 == 0:
                        nr = ns // 2
                        nc.vector.tensor_tensor(
                            out=vs[:, 0:nr, :], in0=vs[:, 0:nr, :],
                            in1=vs[:, nr:ns, :], op=AL.add)
                    red = sm.tile([128, c.C1], F32, tag="red")
                    nc.vector.tensor_reduce(
                        out=red[:], in_=vs[:, 0:nr, :].rearrange("p j c -> p c j"),
                        axis=AX.X, op=AL.add)
                    nc.vector.tensor_tensor(out=num[:], in0=num[:], in1=red[:],
                                            op=AL.add)
                    redd = sm.tile([128, 4], F32, tag="redd")
                    nc.vector.tensor_reduce(
                        out=redd[:],
                        in_=ee[:, :, 0:ns, :].rearrange("p a j h -> p h a j"),
                        axis=AX.XY, op=AL.add)
                    nc.vector.tensor_tensor(out=den[:], in0=den[:], in1=redd[:],
                                            op=AL.add)

                rec = sm.tile([128, 4], F32, tag="rec")
                nc.vector.reciprocal(out=rec[:], in_=den[:])
                h1f = sm.tile([128, c.C1], F32, tag="h1f")
                nc.vector.tensor_tensor(
                    out=h1f[:].rearrange("p (h q) -> p h q", h=4),
                    in0=num[:].rearrange("p (h q) -> p h q", h=4),
                    in1=rec[:, :, None].to_broadcast([128, 4, c.HID]),
                    op=AL.mult)
                h1b = sm.tile([128, c.C1], BF, tag="h1b")
                nc.scalar.activation(out=h1b[:], in_=h1f[:], func=AF.Relu)
                nc.sync.dma_start(out=h1d[b * 128:(b + 1) * 128, :], in_=h1b[:])
                psc = pp.tile([128, 128], F32, tag="C")
                for kh in range(2):
                    ht = pa.tile([128, 128], BF, tag="ht")
                    nc.sync.dma_start(
                        out=ht[:], in_=h1d[b * 128:(b + 1) * 128,
                                           kh * 128:(kh + 1) * 128],
                        transpose=True)
                    nc.tensor.matmul(out=psc[:, 0:66], lhsT=ht[:],
                                     rhs=w2s[:, kh, :],
                                     start=(kh == 0), stop=(kh == 1))
                nc.vector.memset(h2self[:, b, 66:c.T2W], 0.0)
                nc.scalar.activation(out=h2self[:, b, 0:66], in_=psc[:, 0:66],
                                     func=AF.Copy)
                if debug:
                    h1df = sm.tile([128, c.C1], F32, tag="h1df")
                    nc.vector.tensor_relu(out=h1df[:], in_=h1f[:])
                    nc.sync.dma_start(out=h1dbg_d[b * 128:(b + 1) * 128, :],
                                      in_=h1df[:])

            # ============ phase C remainder: ship h2self to DRAM ============
            for g in range(7):
                nc.sync.dma_start(
                    out=cc3in[g * 896:(g + 1) * 896, :].rearrange(
                        "(a p) n -> p a n", p=128),
                    in_=h2self[:, g * 7:(g + 1) * 7, :])

            # ============ phase D: allgather ============
            nc.gpsimd.collective_compute(
                kind="AllGather", op=AL.bypass,
                replica_groups=[list(range(c.NCORES))],
                ins=[cc3in[:, :]], outs=[cc3_rows[:, :]])

            # ============ phase E: layer-2 aggregation ============
            pacc = cpool.tile([128, c.OUT_C], F32)
            nc.vector.memset(pacc[:], 0.0)
            hgt2 = {}
            for b in range(c.NBLK):
                zs2 = sm.tile([128, 1], F32, tag="zs2")
                nc.vector.tensor_tensor(out=zs2[:], in0=h2self[:, b, 64:65],
                                        in1=h2self[:, b, 65:66], op=AL.add)
                lrs2 = sm.tile([128, 1], F32, tag="lrs2")
                nc.vector.scalar_tensor_tensor(
                    out=lrs2[:], in0=zs2[:], scalar=c.NEG, in1=zs2[:],
                    op0=AL.mult, op1=AL.max)
                ees2 = sm.tile([128, 1], BF, tag="ees2")
                nc.scalar.activation(out=ees2[:], in_=lrs2[:], func=AF.Exp)
                num2 = accp.tile([128, c.OUT_C], F32, tag="num2")
                den2 = accp.tile([128, 1], F32, tag="den2")
                nc.scalar.activation(out=den2[:], in_=ees2[:], func=AF.Copy)
                nc.vector.tensor_tensor(
                    out=num2[:], in0=h2self[:, b, 0:c.OUT_C],
                    in1=ees2[:, 0:1].to_broadcast([128, c.OUT_C]), op=AL.mult)

                for (cid, j0, j1) in blk_ranges[b]:
                    ns = j1 - j0
                    c0, c1_ = chunks[cid]
                    ncs = c1_ - c0
                    for g_ in (cid, cid + 1, cid + 2):
                        if g_ < len(chunks) and g_ not in hgt2:
                            g0, g1 = chunks[g_]
                            gn = g1 - g0
                            hgx = pge.tile([128, CS, 2 * c.T2W], BF, tag="hg2")
                            nc.gpsimd.dma_gather(
                                out_ap=hgx[:, 0:gn, :], in_ap=cc3_pairs[:, :],
                                idxs_ap=idx_s[:, g0 * 8:g1 * 8],
                                num_idxs=gn * 128, num_idxs_reg=gn * 128,
                                elem_size=2 * c.T2W, single_packet=False,
                                queue_num=g_ % NQ)
                            hgt2[g_] = hgx
                    hg2 = hgt2[cid]
                    hgr2 = hg2[:, j0:j1, :].rearrange("p j (a w) -> p a j w", a=2)
                    zz = sm.tile([128, 2, CS, 1], F32, tag="zz2")
                    nc.vector.tensor_tensor(
                        out=zz[:, :, 0:ns, :], in0=hgr2[:, :, :, 64:65],
                        in1=h2self[:, b, None, None, 65:66].to_broadcast(
                            [128, 2, ns, 1]), op=AL.add)
                    nc.vector.scalar_tensor_tensor(
                        out=zz[:, :, 0:ns, :], in0=zz[:, :, 0:ns, :], scalar=c.NEG,
                        in1=zz[:, :, 0:ns, :], op0=AL.mult, op1=AL.max)
                    ee = sm.tile([128, 2, CS, 1], BF, tag="ee2")
                    nc.scalar.activation(out=ee[:, :, 0:ns, :],
                                         in_=zz[:, :, 0:ns, :], func=AF.Exp)
                    nc.vector.tensor_tensor(
                        out=ee[:, :, 0:ns, :], in0=ee[:, :, 0:ns, :],
                        in1=par_s[:, :, c0 + j0:c0 + j1, None].to_broadcast(
                            [128, 2, ns, 1]), op=AL.mult)
                    v = pv.tile([128, 2, CS, c.OUT_C], BF, tag="v2")
                    for a in range(2):
                        nc.vector.tensor_tensor(
                            out=v[:, a, 0:ns, :],
                            in0=hg2[:, j0:j1, a * c.T2W:a * c.T2W + c.OUT_C],
                            in1=ee[:, a, 0:ns, 0:1].to_broadcast(
                                [128, ns, c.OUT_C]), op=AL.mult)
                    vs = pv.tile([128, CS, c.OUT_C], BF, tag="v2s")
                    nc.vector.tensor_tensor(
                        out=vs[:, 0:ns, :], in0=v[:, 0, 0:ns, :],
                        in1=v[:, 1, 0:ns, :], op=AL.add)
                    nr = ns
                    if ns % 2 == 0:
                        nr = ns // 2
                        nc.vector.tensor_tensor(
                            out=vs[:, 0:nr, :], in0=vs[:, 0:nr, :],
                            in1=vs[:, nr:ns, :], op=AL.add)
                    red = sm.tile([128, c.OUT_C], F32, tag="red2")
                    nc.vector.tensor_reduce(
                        out=red[:], in_=vs[:, 0:nr, :].rearrange("p j c -> p c j"),
                        axis=AX.X, op=AL.add)
                    nc.vector.tensor_tensor(out=num2[:], in0=num2[:], in1=red[:],
                                            op=AL.add)
                    redd = sm.tile([128, 1], F32, tag="redd2")
                    nc.vector.tensor_reduce(
                        out=redd[:],
                        in_=ee[:, :, 0:ns, :].rearrange("p a j h -> p h a j"),
                        axis=AX.XY, op=AL.add)
                    nc.vector.tensor_tensor(out=den2[:], in0=den2[:], in1=redd[:],
                                            op=AL.add)

                rec2 = sm.tile([128, 1], F32, tag="rec2")
                nc.vector.reciprocal(out=rec2[:], in_=den2[:])
                o2 = sm.tile([128, c.OUT_C], F32, tag="o2")
                nc.vector.tensor_tensor(
                    out=o2[:], in0=num2[:],
                    in1=rec2[:, 0:1].to_broadcast([128, c.OUT_C]), op=AL.mult)
                o2r = sm.tile([128, c.OUT_C], F32, tag="o2r")
                nc.scalar.activation(out=o2r[:], in_=o2[:], func=AF.Relu)
                nc.vector.tensor_tensor(out=pacc[:], in0=pacc[:], in1=o2r[:],
                                        op=AL.add)

            # ============ phase F: pool partial ============
            psf = pp.tile([c.OUT_C + 1, 1], F32, tag="F")
            nc.tensor.matmul(out=psf[0:c.OUT_C, :], lhsT=pacc[:], rhs=ones_s[:],
                             start=True, stop=True)
            pf = sm.tile([c.OUT_C, 1], F32, tag="pf")
            nc.vector.tensor_copy(out=pf[:], in_=psf[0:c.OUT_C, :])
            nc.sync.dma_start(out=pool_d[:, :], in_=pf[:])

    nc.compile()
    legalize_waits(nc)
    return nc


def legalize_waits(nc):
    """Walrus encodes at most ONE sync wait per instruction; hoist extras
    onto same-engine NoOps."""
    for fn in nc.m.functions:
        for bb in fn.blocks:
            insts = list(bb.instructions)
            out = []
            changed = False
            for inst in insts:
                si = inst.sync_info
                if si is not None and si.on_wait and len(si.on_wait) > 1:
                    waits = list(si.on_wait)
                    for w in waits[:-1]:
                        nop = mybir.InstNoOp(
                            name=nc.get_next_instruction_name(), ins=[], outs=[])
                        nop.engine = inst.engine
                        nop.sync_info = mybir.SyncInfo(on_wait=[w], on_update=[])
                        nc.register_instruction(nop)
                        out.append(nop)
                    inst.sync_info = mybir.SyncInfo(
                        on_wait=waits[-1:], on_update=list(si.on_update))
                    changed = True
                out.append(inst)
            if changed:
                bb.instructions.clear()
                bb.instructions.extend(out)


def host_finish(cfg, pools, fc_w, fc_b):
    c = cfg
    tot = np.zeros(c.OUT_C, np.float64)
    for p in pools:
        tot += p[:, 0].astype(np.float64)
    pooled = (tot / c.N).astype(np.float32)
    logits = pooled @ np.asarray(fc_w, np.float32) + np.asarray(fc_b, np.float32)
    m = logits.max()
    ls = logits - (m + np.log(np.exp(logits - m).sum()))
    return ls.reshape(1, c.NCLS).astype(np.float32)


_BUILD_CACHE = {}


def run(cfg, inputs, debug=False, trace=False, **run_kwargs):
    in_maps, meta = host_prep(
        cfg, inputs["x"], inputs["edge_index"], inputs["W1"], inputs["att_src1"],
        inputs["att_dst1"], inputs["b1"], inputs["W2"], inputs["att_src2"],
        inputs["att_dst2"], inputs["b2"])
    key = (meta["SLB"], tuple(meta["Kb"]), debug)
    if key not in _BUILD_CACHE:
        _BUILD_CACHE[key] = build(cfg, meta["SLB"], meta["slab_base"],
                                  debug=debug)
    nc = _BUILD_CACHE[key]
    res = bass_utils.run_bass_kernel_spmd(
        nc, in_maps, core_ids=list(range(cfg.NCORES)), trace=trace, **run_kwargs)
    out = host_finish(cfg, [r["pool64"] for r in res.results],
                      inputs["fc_w"], inputs["fc_b"])
    return out, res


def kernel(**inputs):
    cfg = Cfg()
    out, _ = run(cfg, inputs)
    return out
